# revision 1
# baseline (speedup 1.0000x reference)
"""Trainium2 Bass kernel: CapOnlyContrastiveLoss (margin contrastive loss, mean reduction).

reference math (N=8192, D=512, margin=0.2):
    scores[i,j]  = -||im_i - ex_j||        (via gemm identity)
    diag[i]      = -||im_i - s_i||         (only the diagonal of l2_sim(im, s) is used)
    loss         = mean(relu(margin + scores - diag))

Strategy:
  * 4x2 core grid over (im rows, ex rows): each of the 8 cores handles
    2048 im/s rows x 4096 ex rows -> reads 16 MB (vs 20 MB for the 8x1 hint).
  * PSUM accumulates -2*im.ex (bf16 PE matmul; bf16 input rounding moves
    the final mean by ~1e-6 relative vs the fp32 reference) PLUS exsq[j],
    added as one extra K=128 bf16 matmul per PSUM bank whose rhs rows are
    a hi/lo bf16 split of exsq (error ~4e-3 on d2 ~ 1e3, negligible).
  * Operand transposes ([row,k] -> [k,row]) via XBAR DMA transpose on the
    bf16 casts, SBUF->SBUF.
  * Epilogue is 2 ops per [128, 2048] batch (4 PSUM banks):
      ACT: sq = sqrt(psum + imsq[i])       (activation Sqrt, bias, PSUM read)
      DVE: acc_col = sum_j min(sq, c[i])   (tensor_scalar op0=min with
           fused accumulator, reduce op1=add), using
           relu(c - s) = c - min(s, c), c[i] = margin + ||im_i - s_i||
  * Host finishes: loss = sum_cores(EX_R * sum_i c_i - sum(acc)) / N^2.
"""

import numpy as np

import concourse.bacc as bacc
import concourse.bass as bass
import concourse.tile as tile
from concourse import bass_utils, mybir

N, D = 8192, 512
MARGIN = 0.2
P = 128
NJ = 512  # matmul moving free size (one PSUM bank of fp32)
BJ = 2048  # epilogue batch along j (4 PSUM banks)
I_GROUPS, J_GROUPS = 4, 2  # 8 cores
IM_R = N // I_GROUPS  # 2048 im/s rows per core
EX_R = N // J_GROUPS  # 4096 ex rows per core
KC = D // P  # 4 contraction chunks of 128
N_IT = IM_R // P  # 16 i tiles
N_JB = EX_R // BJ  # 2 big j tiles
BANKS = BJ // NJ  # 4 matmul banks per big tile
EX_PER_JB = BJ // P  # 16 ex row-tiles per big j tile

F32 = mybir.dt.float32
BF16 = mybir.dt.bfloat16
AF = mybir.ActivationFunctionType
ALU = mybir.AluOpType

_CACHE = {}


def _emit(tc, nc, im_d, s_d, ex_d, acc_d, cvec_d):
    from contextlib import ExitStack

    with ExitStack() as ctx:
        singles = ctx.enter_context(tc.tile_pool(name="singles", bufs=1))
        loads = ctx.enter_context(tc.tile_pool(name="loads", bufs=8))
        casts = ctx.enter_context(tc.tile_pool(name="casts", bufs=6))
        scratch = ctx.enter_context(tc.tile_pool(name="scratch", bufs=6))
        epi = ctx.enter_context(tc.tile_pool(name="epi", bufs=6))
        psum = ctx.enter_context(tc.tile_pool(name="psum", bufs=8, space="PSUM"))
        dram = ctx.enter_context(tc.tile_pool(name="dram", bufs=1, space="DRAM"))

        # persistent operands
        imT = singles.tile([P, KC, IM_R], BF16)  # (-2*im)^T
        exTs = [singles.tile([P, KC, BJ], BF16, name=f"exT{j}") for j in range(N_JB)]
        imsq = singles.tile([P, N_IT], F32)
        dd = singles.tile([P, N_IT], F32)
        cc = singles.tile([P, N_IT], F32)
        mv_im = singles.tile([P, N_IT, 2], F32)
        mv_dd = singles.tile([P, N_IT, 2], F32)
        exsq_cols = [singles.tile([P, EX_PER_JB], F32, name=f"exsqc{j}")
                     for j in range(N_JB)]
        acc_sb = singles.tile([P, N_JB * N_IT * BANKS], F32)
        # exsq correction operands: psum[i,j] += 1*hi[j] + 1*lo[j]
        # ones2 rows 0,1 = 1 (rest 0); exrow2 rows 0,1 = hi/lo split of exsq
        ones2 = singles.tile([P, P], BF16)
        exrow2s = [singles.tile([P, BJ], BF16, name=f"exrow2_{j}")
                   for j in range(N_JB)]
        exrow2_dram = dram.tile([2, EX_R], BF16)

        nc.vector.memset(ones2, 0.0)
        nc.vector.memset(ones2[0:2, :], 1.0)
        for jb in range(N_JB):
            nc.vector.memset(exrow2s[jb], 0.0)

        def emit_im_tile(t):
            im_t = loads.tile([P, D], F32, tag="im_t")
            s_t = loads.tile([P, D], F32, tag="s_t")
            nc.sync.dma_start(out=im_t, in_=im_d[t * P:(t + 1) * P, :])
            nc.sync.dma_start(out=s_t, in_=s_d[t * P:(t + 1) * P, :])
            # rowwise stats via bn_stats: sum(x^2) = D*(var + mean^2)
            st1 = scratch.tile([P, 6], F32, tag="st1")
            nc.vector.bn_stats(out=st1, in_=im_t)
            nc.vector.bn_aggr(out=mv_im[:, t, :], in_=st1)
            diff = scratch.tile([P, D], F32, tag="diff")
            nc.gpsimd.tensor_tensor(out=diff, in0=im_t, in1=s_t, op=ALU.subtract)
            st2 = scratch.tile([P, 6], F32, tag="st2")
            nc.vector.bn_stats(out=st2, in_=diff)
            nc.vector.bn_aggr(out=mv_dd[:, t, :], in_=st2)
            imb = casts.tile([P, D], BF16, tag="imb")
            nc.vector.tensor_scalar_mul(imb, im_t, -2.0)
            nc.sync.dma_start_transpose(imT[:, :, t * P:(t + 1) * P], imb)

        def emit_ex_tile(t):
            jb, u = divmod(t, EX_PER_JB)
            ex_t = loads.tile([P, D], F32, tag="ex_t")
            nc.sync.dma_start(out=ex_t, in_=ex_d[t * P:(t + 1) * P, :])
            sqo = scratch.tile([P, D], F32, tag="sqo")
            nc.scalar.activation(out=sqo, in_=ex_t, func=AF.Square,
                                 accum_out=exsq_cols[jb][:, u:u + 1])
            exb = casts.tile([P, D], BF16, tag="exb")
            nc.vector.tensor_copy(out=exb, in_=ex_t)
            nc.sync.dma_start_transpose(exTs[jb][:, :, u * P:(u + 1) * P], exb)

        # interleave im and ex preamble so both stream concurrently
        for t in range(N_IT):
            emit_im_tile(t)
            emit_ex_tile(2 * t)
            emit_ex_tile(2 * t + 1)

        # imsq = D*(var + mean^2), dd likewise; c = margin + sqrt(dd)
        for mv, col in ((mv_im, imsq), (mv_dd, dd)):
            nc.vector.tensor_tensor(out=col, in0=mv[:, :, 0], in1=mv[:, :, 0],
                                    op=ALU.mult)
            nc.vector.tensor_tensor(out=col, in0=col, in1=mv[:, :, 1], op=ALU.add)
            nc.vector.tensor_scalar_mul(col, col, float(D))
        nc.scalar.activation(out=cc, in_=dd, func=AF.Sqrt)
        nc.vector.tensor_scalar_add(cc, cc, MARGIN)
        nc.sync.dma_start(out=cvec_d, in_=cc)

        # hi/lo bf16 split of exsq, scattered to j-order rows in DRAM, then
        # loaded back as rows 0,1 of exrow2.
        for jb in range(N_JB):
            hi = scratch.tile([P, EX_PER_JB], BF16, tag="hi")
            lo = scratch.tile([P, EX_PER_JB], BF16, tag="lo")
            nc.vector.tensor_copy(out=hi, in_=exsq_cols[jb])
            nc.vector.tensor_tensor(out=lo, in0=exsq_cols[jb], in1=hi,
                                    op=ALU.subtract)
            sl = slice(jb * BJ, (jb + 1) * BJ)
            nc.sync.dma_start(
                out=exrow2_dram[0:1, sl].rearrange("o (u p) -> (o p) u", p=P), in_=hi)
            nc.sync.dma_start(
                out=exrow2_dram[1:2, sl].rearrange("o (u p) -> (o p) u", p=P), in_=lo)
            nc.sync.dma_start(out=exrow2s[jb][0:2, :], in_=exrow2_dram[:, sl])

        # ---- main loop: 128 psum tiles of [128 i, 512 j], 8-deep pipeline ----
        for jb in range(N_JB):
            for it in range(N_IT):
                for b in range(BANKS):
                    ps = psum.tile([P, NJ], F32, tag="mm")
                    for k in range(KC):
                        nc.tensor.matmul(ps,
                                         imT[:, k, it * P:(it + 1) * P],
                                         exTs[jb][:, k, b * NJ:(b + 1) * NJ],
                                         start=(k == 0), stop=False)
                    # += exsq[j]  (hi + lo rows; rows 2..127 are zero)
                    nc.tensor.matmul(
                        ps, ones2,
                        exrow2s[jb][:, b * NJ:(b + 1) * NJ],
                        start=False, stop=True)
                    # sq = sqrt(d2 + imsq[i])   (ACT, PSUM -> SBUF)
                    sq = epi.tile([P, NJ], F32, tag="sq")
                    nc.scalar.activation(out=sq, in_=ps, func=AF.Sqrt,
                                         bias=imsq[:, it:it + 1], scale=1.0)
                    # acc[:, col] = sum_j min(sq, c)   (DVE, fused accumulate)
                    col = (jb * N_IT + it) * BANKS + b
                    nc.vector.tensor_scalar(sq, sq, cc[:, it:it + 1], 0.0,
                                            ALU.min, ALU.add,
                                            accum_out=acc_sb[:, col:col + 1])

        nc.sync.dma_start(out=acc_d, in_=acc_sb)


def build_program():
    nc = bacc.Bacc("TRN2", target_bir_lowering=False, debug=False)
    im_d = nc.dram_tensor("im", [IM_R, D], F32, kind="ExternalInput").ap()
    s_d = nc.dram_tensor("s", [IM_R, D], F32, kind="ExternalInput").ap()
    ex_d = nc.dram_tensor("ex", [EX_R, D], F32, kind="ExternalInput").ap()
    acc_d = nc.dram_tensor("acc", [P, N_JB * N_IT * BANKS], F32, kind="ExternalOutput").ap()
    cvec_d = nc.dram_tensor("cvec", [P, N_IT], F32, kind="ExternalOutput").ap()
    with tile.TileContext(nc) as tc:
        _emit(tc, nc, im_d, s_d, ex_d, acc_d, cvec_d)
    nc.compile()
    return nc


def get_program():
    if "nc" not in _CACHE:
        _CACHE["nc"] = build_program()
    return _CACHE["nc"]


def make_in_maps(im, s, ex_s):
    in_maps = []
    for c in range(8):
        ig, jg = divmod(c, J_GROUPS)
        in_maps.append({
            "im": np.ascontiguousarray(im[ig * IM_R:(ig + 1) * IM_R], dtype=np.float32),
            "s": np.ascontiguousarray(s[ig * IM_R:(ig + 1) * IM_R], dtype=np.float32),
            "ex": np.ascontiguousarray(ex_s[jg * EX_R:(jg + 1) * EX_R], dtype=np.float32),
        })
    return in_maps


def finish(results):
    # per core: sum_ij relu(c_i - sq_ij) = EX_R * sum_i c_i - sum_ij min(sq, c)
    total = 0.0
    for r in results:
        total += float(EX_R) * float(np.sum(r["cvec"], dtype=np.float64))
        total -= float(np.sum(r["acc"], dtype=np.float64))
    return np.array(total / (float(N) * float(N)), dtype=np.float32)


def kernel(im, s, ex_s):
    nc = get_program()
    res = bass_utils.run_bass_kernel_spmd(nc, make_in_maps(im, s, ex_s),
                                          core_ids=list(range(8)))
    return finish(res.results)


if __name__ == "__main__":
    rng = np.random.default_rng(0)
    im = rng.standard_normal((N, D), dtype=np.float32)
    s = rng.standard_normal((N, D), dtype=np.float32)
    ex = rng.standard_normal((N, D), dtype=np.float32)
    print(kernel(im, s, ex))

